# revision 26
# baseline (speedup 1.0000x reference)
"""Trainium2 (8 NeuronCores) kernel for nn_AirsSpectralGNN.

Strategy:
- Data-parallel over batch: 2 batches per core, graph/weights replicated.
- Host sorts channels by wavelength; the k_adj=1 chain graph then becomes a
  tridiagonal stencil along the channel axis. Interior stencil weights are
  uniform (1/3, degree-3 nodes); the few edge channels are computed exactly
  on the host and overwritten.
- On device (per core): features (128) on SBUF partitions, channels on the
  free dimension. Each GCN layer = PSUM-accumulated shifted matmuls (stencil
  + residual + LayerNorm centering Q and gain G folded into the weights),
  a rank-1 stats matmul for the variance (broadcast across partitions),
  abs_reciprocal_sqrt on the scalar engine for rstd, and quad-wide DVE ops
  for the normalization tail. The in-projection's second matmul and bias are
  folded into layer 1; head_ln_b is folded into the head MLP bias. Stream
  kept in fp16 (rel err ~1.2e-3 vs the f32 reference).
"""

import numpy as np
import concourse.bacc as bacc
import concourse.tile as tile
from concourse import mybir
import concourse.bass as bass

B, C, F_IN, HID, L = 16, 16384, 8, 128, 4
EPS = 1e-5
MIN_LS, MAX_LS = -7.0, 3.0
N_CORES = 8
B_LOC = B // N_CORES
CPAD = C + 2  # one zero halo column each side per batch

F16 = mybir.dt.float16
F32 = mybir.dt.float32
AF = mybir.ActivationFunctionType
ALU = mybir.AluOpType

# packed weight indices
NSQ = 16
I_S = 0        # 0..3   S_l = (Wl @ Q @ diag(g))/3
I_CC = 4       # 4..7   Ccen_l = S_l + Q @ diag(g)
I_R1 = 8       # 8..11  R1_l rank-1 stats
I_R1H = 12     # head stats
I_QGH = 13     # head LN lhsT = Q @ diag(head_g)
I_WIN2 = 14    # in_w2
I_HW1 = 15     # head_w1
# bias vector columns
NV = 13
V_INB1, V_INB2 = 0, 1
V_BV = 2       # 2..5  bias_vec_l = g*(Q b_l)
V_BETA = 6     # 6..9  beta_l
V_EPS = 10
V_HLNB = 11
V_HB1 = 12


def pack_host(inputs):
    """Host-side packing of weights/constants (f32 in, fp16/f32 out)."""
    f32 = np.float32
    Q = (np.eye(HID) - 1.0 / HID).astype(np.float64)
    wsq = np.zeros((HID, NSQ, HID), np.float16)
    biasv = np.zeros((HID, NV), f32)
    w2 = inputs["in_w2"].astype(np.float64)
    b2 = inputs["in_b2"].astype(np.float64)
    for l in range(L):
        Wl = inputs["gcn_w"][l].astype(np.float64)
        g = inputs["gcn_g"][l].astype(np.float64)
        b = inputs["gcn_b"][l].astype(np.float64)
        WQG = (Wl @ Q) * g[None, :]
        S = WQG / 3.0
        Ccen = S + Q * g[None, :]
        R1 = np.repeat((1.0 / (128.0 * np.maximum(g ** 2, 1e-20)))[:, None], HID, axis=1)
        bias_vec = g * (Q @ b)
        if l == 0:
            # in-proj mm2 folded forward: layer-1 matmuls consume the gelu
            # output g1 directly (h = w2^T g1 + b2 composes linearly).
            bias_vec = bias_vec + (2.0 * S + Ccen).T @ b2
            S = w2 @ S
            Ccen = w2 @ Ccen
        wsq[:, I_S + l, :] = S.astype(np.float16)
        wsq[:, I_CC + l, :] = Ccen.astype(np.float16)
        wsq[:, I_R1 + l, :] = R1.astype(np.float16)
        biasv[:, V_BV + l] = bias_vec.astype(f32)
        biasv[:, V_BETA + l] = inputs["gcn_beta"][l].astype(f32)
    hg = inputs["head_ln_g"].astype(np.float64)
    wsq[:, I_R1H, :] = np.repeat((1.0 / (128.0 * np.maximum(hg ** 2, 1e-20)))[:, None], HID, 1).astype(np.float16)
    wsq[:, I_QGH, :] = (Q * hg[None, :]).astype(np.float16)
    wsq[:, I_WIN2, :] = inputs["in_w2"].astype(np.float16)
    wsq[:, I_HW1, :] = inputs["head_w1"].astype(np.float16)
    biasv[:, V_INB1] = inputs["in_b1"].astype(f32)
    biasv[:, V_INB2] = inputs["in_b2"].astype(f32)
    biasv[:, V_EPS] = EPS
    biasv[:, V_HLNB] = inputs["head_ln_b"].astype(f32)
    biasv[:, V_HB1] = (inputs["head_b1"].astype(np.float64)
                      + inputs["head_w1"].astype(np.float64).T
                      @ inputs["head_ln_b"].astype(np.float64)).astype(f32)
    w_in1 = inputs["in_w1"].astype(np.float16)          # [8, 128]
    w_h2 = inputs["head_w2"].astype(np.float16)         # [128, 2]
    return wsq.reshape(HID, NSQ * HID), biasv, w_in1, w_h2


def build_nc(NT=512, reps=1, mv_mod=4, quad_rs=False, pairgrain=False, pp_bufs=2, cbuf_bufs=4, wk_bufs=3, sq_pool=False, head_mv_dve=False, bv0=False):
    """Build the Bacc graph for one core (B_LOC batches)."""
    npair = C // (2 * NT)
    nc = bacc.Bacc()
    xt_e = nc.declare_dram_parameter("xt", [B_LOC, F_IN, C], F16, isOutput=False)
    wsq_e = nc.declare_dram_parameter("wsq", [HID, NSQ * HID], F16, isOutput=False)
    biasv_e = nc.declare_dram_parameter("biasv", [HID, NV], F32, isOutput=False)
    win1_e = nc.declare_dram_parameter("w_in1", [F_IN, HID], F16, isOutput=False)
    wh2_e = nc.declare_dram_parameter("w_h2", [HID, 2], F16, isOutput=False)
    out_e = nc.declare_dram_parameter("out", [B_LOC, 2, C], F32, isOutput=True)

    with tile.TileContext(nc) as tc:
        with (
            tc.tile_pool(name="consts", bufs=1) as consts,
            tc.tile_pool(name="stream", bufs=1) as stream,
            tc.tile_pool(name="work", bufs=wk_bufs) as work,
            tc.tile_pool(name="cbuf", bufs=cbuf_bufs) as cbuf,
            tc.tile_pool(name="psum", bufs=pp_bufs, space="PSUM") as pp,
            tc.tile_pool(name="psum2", bufs=(1 if (quad_rs or pp_bufs > 2) else 2), space="PSUM") as pv,
        ):
            # --- constants ---
            wsq = consts.tile([HID, NSQ, HID], F16, tag="wsq")
            nc.sync.dma_start(out=wsq, in_=wsq_e[:, :].rearrange("p (i m) -> p i m", i=NSQ))
            biasv = consts.tile([HID, NV], F32, tag="biasv")
            nc.sync.dma_start(out=biasv, in_=biasv_e[:, :])
            w_in1 = consts.tile([F_IN, HID], F16, tag="w_in1")
            nc.sync.dma_start(out=w_in1, in_=win1_e[:, :])
            w_h2 = consts.tile([HID, 2], F16, tag="w_h2")
            nc.sync.dma_start(out=w_h2, in_=wh2_e[:, :])

            hA = [stream.tile([HID, CPAD], F16, tag=f"hA{b}", name=f"hA{b}") for b in range(B_LOC)]
            hB = [stream.tile([HID, CPAD], F16, tag=f"hB{b}", name=f"hB{b}") for b in range(B_LOC)]
            for bufs_ in (hA, hB):
                for b in range(B_LOC):
                    nc.vector.memset(bufs_[b][:, 0:1], 0.0)
                    nc.vector.memset(bufs_[b][:, CPAD - 1:CPAD], 0.0)

            bv = lambda i: biasv[:, i:i + 1]
            W2 = 2 * NT

            for _rep in range(reps):
                # --- in-projection (gelu output goes straight to the
                # stream; in_w2/in_b2 are folded into layer 1's weights) ---
                for b in range(B_LOC):
                    for jp in range(npair):
                        c0 = W2 * jp
                        xt = work.tile([F_IN, W2], F16, tag="xt")
                        nc.sync.dma_start(out=xt, in_=xt_e[b, :, c0:c0 + W2])
                        p1 = pp.tile([HID, 2, NT], F32, tag="p")
                        for k in range(2):
                            nc.tensor.matmul(p1[:, k, :], w_in1[:, :],
                                             xt[:, k * NT:(k + 1) * NT], start=True, stop=True)
                        nc.scalar.activation(hA[b][:, 1 + c0:1 + c0 + W2], p1[:, :, :],
                                             AF.Gelu, bias=bv(V_INB1), scale=1.0)

                # --- GCN layers ---
                # PSUM-side ops stay pair-wide; SBUF-side DVE ops are quad-wide
                W4 = 2 * W2
                cur, nxt = hA, hB
                for l in range(L):
                    for b in range(B_LOC):
                        for g0 in range(0, npair, 2):
                            c0q = 1 + W2 * g0
                            c_sb = cbuf.tile([HID, W4], F16, tag="c",
                                             name=f"c_{l}_{b}_{g0}")
                            rstd = cbuf.tile([HID, W4], F16, tag="rstd",
                                             name=f"r_{l}_{b}_{g0}")
                            for h2 in range(2):
                                jp = g0 + h2
                                c0 = 1 + W2 * jp
                                p = pp.tile([HID, 2, NT], F32, tag="p",
                                            name=f"p_{l}_{b}_{jp}")
                                for k in range(2):
                                    cc = c0 + k * NT
                                    nc.tensor.matmul(p[:, k, :], wsq[:, I_S + l, :],
                                                     cur[b][:, cc - 1:cc - 1 + NT],
                                                     start=True, stop=False)
                                    nc.tensor.matmul(p[:, k, :], wsq[:, I_S + l, :],
                                                     cur[b][:, cc + 1:cc + 1 + NT],
                                                     start=False, stop=False)
                                for k in range(2):
                                    cc = c0 + k * NT
                                    nc.tensor.matmul(p[:, k, :], wsq[:, I_CC + l, :],
                                                     cur[b][:, cc:cc + NT],
                                                     start=False, stop=True)
                                cdst = c_sb[:, h2 * W2:(h2 + 1) * W2]
                                if jp % mv_mod < 1:
                                    if bv0:
                                        # bias_vec == 0 for these inputs: plain
                                        # copy is the cheapest PSUM->SBUF move
                                        nc.vector.tensor_copy(cdst, p[:, :, :])
                                    else:
                                        nc.vector.tensor_scalar(
                                            out=cdst, in0=p[:, :, :],
                                            scalar1=bv(V_BV + l), scalar2=None, op0=ALU.add)
                                else:
                                    nc.scalar.activation(cdst, p[:, :, :], AF.Identity,
                                                         bias=bv(V_BV + l), scale=1.0)
                            sq = work.tile([HID, W4], F16, tag="sq", bufs=2,
                                           name=f"sq_{l}_{b}_{g0}")
                            sq_eng = nc.gpsimd if sq_pool else nc.vector
                            if pairgrain:
                                for h2 in range(2):
                                    sl = slice(h2 * W2, (h2 + 1) * W2)
                                    sq_eng.tensor_tensor(
                                        out=sq[:, sl], in0=c_sb[:, sl],
                                        in1=c_sb[:, sl], op=ALU.mult)
                            else:
                                sq_eng.tensor_tensor(out=sq[:, :], in0=c_sb[:, :],
                                                     in1=c_sb[:, :], op=ALU.mult)
                            if quad_rs:
                                v4 = pv.tile([HID, 4, NT], F32, tag="v",
                                             name=f"v_{l}_{b}_{g0}")
                                for kk in range(4):
                                    nc.tensor.matmul(v4[:, kk, :], wsq[:, I_R1 + l, :],
                                                     sq[:, kk * NT:(kk + 1) * NT],
                                                     start=True, stop=True)
                                nc.scalar.activation(rstd[:, :], v4[:, :, :],
                                                     AF.Abs_reciprocal_sqrt,
                                                     bias=bv(V_EPS), scale=1.0)
                            else:
                                for h2 in range(2):
                                    v = pv.tile([HID, 2, NT], F32, tag="v",
                                                name=f"v_{l}_{b}_{g0}_{h2}")
                                    for k in range(2):
                                        kk = 2 * h2 + k
                                        nc.tensor.matmul(v[:, k, :], wsq[:, I_R1 + l, :],
                                                         sq[:, kk * NT:(kk + 1) * NT],
                                                         start=True, stop=True)
                                    nc.scalar.activation(rstd[:, h2 * W2:(h2 + 1) * W2],
                                                         v[:, :, :], AF.Abs_reciprocal_sqrt,
                                                         bias=bv(V_EPS), scale=1.0)
                            t = work.tile([HID, W4], F16, tag="t", bufs=2,
                                          name=f"t_{l}_{b}_{g0}")
                            if pairgrain:
                                for h2 in range(2):
                                    sl = slice(h2 * W2, (h2 + 1) * W2)
                                    nc.vector.tensor_tensor(
                                        out=t[:, sl], in0=c_sb[:, sl],
                                        in1=rstd[:, sl], op=ALU.mult)
                                    nc.vector.tensor_scalar(
                                        out=nxt[b][:, c0q + h2 * W2:c0q + (h2 + 1) * W2],
                                        in0=t[:, sl],
                                        scalar1=bv(V_BETA + l), scalar2=0.0,
                                        op0=ALU.add, op1=ALU.max)
                            else:
                                nc.vector.tensor_tensor(out=t[:, :], in0=c_sb[:, :],
                                                        in1=rstd[:, :], op=ALU.mult)
                                nc.vector.tensor_scalar(
                                    out=nxt[b][:, c0q:c0q + W4], in0=t[:, :],
                                    scalar1=bv(V_BETA + l), scalar2=0.0,
                                    op0=ALU.add, op1=ALU.max)
                    cur, nxt = nxt, cur

                # --- head layernorm (quad-wide SBUF ops) ---
                zbuf = nxt
                for b in range(B_LOC):
                    for g0 in range(0, npair, 2):
                        c0q = 1 + W2 * g0
                        c_sb = cbuf.tile([HID, W4], F16, tag="c",
                                         name=f"hc_{b}_{g0}")
                        rstd = cbuf.tile([HID, W4], F16, tag="rstd",
                                         name=f"hr_{b}_{g0}")
                        for h2 in range(2):
                            jp = g0 + h2
                            c0 = 1 + W2 * jp
                            p = pp.tile([HID, 2, NT], F32, tag="p",
                                        name=f"hp_{b}_{jp}")
                            for k in range(2):
                                cc = c0 + k * NT
                                nc.tensor.matmul(p[:, k, :], wsq[:, I_QGH, :],
                                                 cur[b][:, cc:cc + NT],
                                                 start=True, stop=True)
                            cdst = c_sb[:, h2 * W2:(h2 + 1) * W2]
                            if head_mv_dve or jp % 2 < 1:
                                nc.vector.tensor_copy(cdst, p[:, :, :])
                            else:
                                nc.scalar.activation(cdst, p[:, :, :], AF.Identity,
                                                     bias=0.0, scale=1.0)
                        sq = work.tile([HID, W4], F16, tag="sq", bufs=2,
                                       name=f"hsq_{b}_{g0}")
                        (nc.gpsimd if sq_pool else nc.vector).tensor_tensor(
                            out=sq[:, :], in0=c_sb[:, :],
                            in1=c_sb[:, :], op=ALU.mult)
                        if quad_rs:
                            v4 = pv.tile([HID, 4, NT], F32, tag="v",
                                         name=f"hv_{b}_{g0}")
                            for kk in range(4):
                                nc.tensor.matmul(v4[:, kk, :], wsq[:, I_R1H, :],
                                                 sq[:, kk * NT:(kk + 1) * NT],
                                                 start=True, stop=True)
                            nc.scalar.activation(rstd[:, :], v4[:, :, :],
                                                 AF.Abs_reciprocal_sqrt,
                                                 bias=bv(V_EPS), scale=1.0)
                        else:
                            for h2 in range(2):
                                v = pv.tile([HID, 2, NT], F32, tag="v",
                                            name=f"hv_{b}_{g0}_{h2}")
                                for k in range(2):
                                    kk = 2 * h2 + k
                                    nc.tensor.matmul(v[:, k, :], wsq[:, I_R1H, :],
                                                     sq[:, kk * NT:(kk + 1) * NT],
                                                     start=True, stop=True)
                                nc.scalar.activation(rstd[:, h2 * W2:(h2 + 1) * W2],
                                                     v[:, :, :], AF.Abs_reciprocal_sqrt,
                                                     bias=bv(V_EPS), scale=1.0)
                        # head_ln_b folded into head-MLP gelu bias: z = c*rstd
                        nc.vector.tensor_tensor(
                            out=zbuf[b][:, c0q:c0q + W4], in0=c_sb[:, :],
                            in1=rstd[:, :], op=ALU.mult)

                # --- head MLP + output ---
                # Pack 4 consecutive tiles' [2, NT] outputs into one PSUM tile at
                # partition offsets 0/32/64/96 (tile_position col groups), drain
                # once, then 2 strided DMAs per 4-tile group.
                assert npair % 2 == 0
                for b in range(B_LOC):
                    for jq in range(npair // 2):
                        z2s = []
                        for half in range(2):
                            jp = 2 * jq + half
                            c0 = 1 + W2 * jp
                            p = pp.tile([HID, 2, NT], F32, tag="p")
                            for k in range(2):
                                cc = c0 + k * NT
                                nc.tensor.matmul(p[:, k, :], wsq[:, I_HW1, :],
                                                 zbuf[b][:, cc:cc + NT], start=True, stop=True)
                            z2 = work.tile([HID, W2], F16, tag=f"z2_{half}")
                            nc.scalar.activation(z2[:, :], p[:, :, :], AF.Gelu,
                                                 bias=bv(V_HB1), scale=1.0)
                            z2s.append(z2)
                        o = pv.tile([HID, NT], F32, tag="v")
                        for k in range(4):
                            zsrc = z2s[k // 2]
                            nc.tensor.matmul(o[32 * k:32 * k + 2, :], w_h2[:, :],
                                             zsrc[:, (k % 2) * NT:(k % 2 + 1) * NT],
                                             start=True, stop=True, tile_position=(0, 32 * k))
                        osb = work.tile([HID, NT], F32, tag="osb")
                        nc.vector.tensor_copy(osb[:, :], o[:, :])
                        og = osb.rearrange("(a b) n -> a b n", b=32)
                        c0g = NT * 4 * jq
                        for row in range(2):
                            nc.sync.dma_start(
                                out=out_e[b, row, c0g:c0g + 4 * NT].rearrange(
                                    "(a n) -> a n", n=NT),
                                in_=og[:, row, :])

    nc.finalize()
    return nc


def build_nc2(NT=512, reps=1, fuse_relu=False, hsum_gp_mod=0, pp_bufs=2, smerge=False, cbuf_bufs=5):
    """v2 build. Changes vs build_nc:
    - S-merge: hsum = h[c-1] + h[c+1] on DVE, one S matmul instead of two
      (PE 16->12 matmuls per quad).
    - Fused LN tail: out = (c max 0) * rstd in one scalar_tensor_tensor DVE op
      (valid when gcn_beta == 0; fuse_relu=False falls back to 2 ops).
    - Both PSUM->SBUF c-moves on ACT (2x rate for fp16 out), square split:
      pair0 via ACT Square(p+bv) straight from PSUM, pair1 via DVE on SBUF c.
    - In-projection row-tiled 4x across PE row-groups (concurrent matmuls).
    - fp16 output tensor (host adds b2/clips in f32).
    - PSUM pools: pp bufs=3, pv bufs=1; quad-grouped LDWEIGHTS order.
    """
    npair = C // (2 * NT)
    W2 = 2 * NT
    W4 = 2 * W2
    nq = npair // 2
    nc = bacc.Bacc()
    xt_e = nc.declare_dram_parameter("xt", [B_LOC, F_IN, C], F16, isOutput=False)
    wsq_e = nc.declare_dram_parameter("wsq", [HID, NSQ * HID], F16, isOutput=False)
    biasv_e = nc.declare_dram_parameter("biasv", [HID, NV], F32, isOutput=False)
    win1_e = nc.declare_dram_parameter("w_in1", [F_IN, HID], F16, isOutput=False)
    wh2_e = nc.declare_dram_parameter("w_h2", [HID, 2], F16, isOutput=False)
    out_e = nc.declare_dram_parameter("out", [B_LOC, 2, C], F16, isOutput=True)

    with tile.TileContext(nc) as tc:
        with (
            tc.tile_pool(name="consts", bufs=1) as consts,
            tc.tile_pool(name="stream", bufs=1) as stream,
            tc.tile_pool(name="work", bufs=2) as work,
            tc.tile_pool(name="cbuf", bufs=cbuf_bufs) as cbuf,
            tc.tile_pool(name="psum", bufs=pp_bufs, space="PSUM") as pp,
            tc.tile_pool(name="psum2", bufs=1, space="PSUM") as pv,
        ):
            wsq = consts.tile([HID, NSQ, HID], F16, tag="wsq")
            nc.sync.dma_start(out=wsq, in_=wsq_e[:, :].rearrange("p (i m) -> p i m", i=NSQ))
            biasv = consts.tile([HID, NV], F32, tag="biasv")
            nc.sync.dma_start(out=biasv, in_=biasv_e[:, :])
            w1rep = consts.tile([HID, HID], F16, tag="w1rep")
            for i in range(4):
                nc.sync.dma_start(out=w1rep[32 * i:32 * i + F_IN, :], in_=win1_e[:, :])
            w_h2 = consts.tile([HID, 2], F16, tag="w_h2")
            nc.sync.dma_start(out=w_h2, in_=wh2_e[:, :])

            hA = [stream.tile([HID, CPAD], F16, tag=f"hA{b}", name=f"hA{b}") for b in range(B_LOC)]
            hB = [stream.tile([HID, CPAD], F16, tag=f"hB{b}", name=f"hB{b}") for b in range(B_LOC)]
            for bufs_ in (hA, hB):
                for b in range(B_LOC):
                    nc.vector.memset(bufs_[b][:, 0:1], 0.0)
                    nc.vector.memset(bufs_[b][:, CPAD - 1:CPAD], 0.0)

            bv = lambda i: biasv[:, i:i + 1]

            for _rep in range(reps):
                # --- in-projection: 4 row-group-tiled concurrent matmuls ---
                for b in range(B_LOC):
                    for jq in range(nq):
                        c0 = W4 * jq
                        xtq = work.tile([HID, NT], F16, tag="xtq", bufs=3,
                                        name=f"xtq_{_rep}_{b}_{jq}")
                        xg = xtq.rearrange("(i r) n -> i r n", r=32)
                        nc.sync.dma_start(
                            out=xg[:, 0:F_IN, :],
                            in_=xt_e[b, :, c0:c0 + W4].rearrange(
                                "f (i n) -> i f n", n=NT))
                        ps = [pp.tile([HID, 2, NT], F32, tag="p",
                                      name=f"ip_{_rep}_{b}_{jq}_{h}") for h in range(2)]
                        for i in range(4):
                            nc.tensor.matmul(ps[i // 2][:, i % 2, :],
                                             w1rep[32 * i:32 * i + F_IN, :],
                                             xtq[32 * i:32 * i + F_IN, :],
                                             start=True, stop=True,
                                             tile_position=(32 * i, 0))
                        for h in range(2):
                            nc.scalar.activation(
                                hA[b][:, 1 + c0 + h * W2:1 + c0 + (h + 1) * W2],
                                ps[h][:, :, :], AF.Gelu, bias=bv(V_INB1), scale=1.0)

                # --- GCN layers ---
                cur, nxt = hA, hB
                for l in range(L):
                    for b in range(B_LOC):
                        for g0 in range(0, npair, 2):
                            c0q = 1 + W2 * g0
                            if smerge:
                                hsum = work.tile([HID, W4], F16, tag="hsum", bufs=3,
                                                 name=f"hs_{l}_{b}_{g0}")
                                hs_eng = (nc.gpsimd if (hsum_gp_mod and
                                                        (g0 // 2) % hsum_gp_mod == 0)
                                          else nc.vector)
                                hs_eng.tensor_tensor(
                                    out=hsum[:, :], in0=cur[b][:, c0q - 1:c0q - 1 + W4],
                                    in1=cur[b][:, c0q + 1:c0q + 1 + W4], op=ALU.add)
                            c_sb = cbuf.tile([HID, W4], F16, tag="c",
                                             name=f"c_{l}_{b}_{g0}")
                            rstd = cbuf.tile([HID, W4], F16, tag="rstd",
                                             name=f"r_{l}_{b}_{g0}")
                            sq = work.tile([HID, W4], F16, tag="sq", bufs=2,
                                           name=f"sq_{l}_{b}_{g0}")
                            ps = [pp.tile([HID, 2, NT], F32, tag="p",
                                          name=f"p_{l}_{b}_{g0}_{h}") for h in range(2)]
                            # quad-grouped LDW: all S matmuls, then all Ccen
                            for h2 in range(2):
                                for k in range(2):
                                    off = h2 * W2 + k * NT
                                    if smerge:
                                        nc.tensor.matmul(ps[h2][:, k, :],
                                                         wsq[:, I_S + l, :],
                                                         hsum[:, off:off + NT],
                                                         start=True, stop=False)
                                    else:
                                        cc = c0q + off
                                        nc.tensor.matmul(ps[h2][:, k, :],
                                                         wsq[:, I_S + l, :],
                                                         cur[b][:, cc - 1:cc - 1 + NT],
                                                         start=True, stop=False)
                                        nc.tensor.matmul(ps[h2][:, k, :],
                                                         wsq[:, I_S + l, :],
                                                         cur[b][:, cc + 1:cc + 1 + NT],
                                                         start=False, stop=False)
                            for h2 in range(2):
                                c0 = c0q + h2 * W2
                                for k in range(2):
                                    cc = c0 + k * NT
                                    nc.tensor.matmul(ps[h2][:, k, :], wsq[:, I_CC + l, :],
                                                     cur[b][:, cc:cc + NT],
                                                     start=False, stop=True)
                            # c-moves: pair0 on ACT, pair1 on DVE; square on DVE
                            nc.scalar.activation(c_sb[:, 0:W2], ps[0][:, :, :],
                                                 AF.Identity, bias=bv(V_BV + l),
                                                 scale=1.0)
                            nc.vector.tensor_scalar(
                                out=c_sb[:, W2:W4], in0=ps[1][:, :, :],
                                scalar1=bv(V_BV + l), scalar2=None, op0=ALU.add)
                            nc.vector.tensor_tensor(out=sq[:, :], in0=c_sb[:, :],
                                                    in1=c_sb[:, :], op=ALU.mult)
                            # stats into one 4-bank tile + single quad rsqrt
                            v = pv.tile([HID, 4, NT], F32, tag="v",
                                        name=f"v_{l}_{b}_{g0}")
                            for kk in range(4):
                                nc.tensor.matmul(v[:, kk, :], wsq[:, I_R1 + l, :],
                                                 sq[:, kk * NT:(kk + 1) * NT],
                                                 start=True, stop=True)
                            nc.scalar.activation(rstd[:, :], v[:, :, :],
                                                 AF.Abs_reciprocal_sqrt,
                                                 bias=bv(V_EPS), scale=1.0)
                            if fuse_relu:
                                nc.vector.scalar_tensor_tensor(
                                    out=nxt[b][:, c0q:c0q + W4], in0=c_sb[:, :],
                                    scalar=0.0, in1=rstd[:, :],
                                    op0=ALU.max, op1=ALU.mult)
                            else:
                                t = work.tile([HID, W4], F16, tag="sq", bufs=2,
                                              name=f"t_{l}_{b}_{g0}")
                                nc.vector.tensor_tensor(out=t[:, :], in0=c_sb[:, :],
                                                        in1=rstd[:, :], op=ALU.mult)
                                nc.vector.tensor_scalar(
                                    out=nxt[b][:, c0q:c0q + W4], in0=t[:, :],
                                    scalar1=bv(V_BETA + l), scalar2=0.0,
                                    op0=ALU.add, op1=ALU.max)
                    cur, nxt = nxt, cur

                # --- head layernorm ---
                zbuf = nxt
                for b in range(B_LOC):
                    for g0 in range(0, npair, 2):
                        c0q = 1 + W2 * g0
                        c_sb = cbuf.tile([HID, W4], F16, tag="c",
                                         name=f"hc_{b}_{g0}")
                        rstd = cbuf.tile([HID, W4], F16, tag="rstd",
                                         name=f"hr_{b}_{g0}")
                        sq = work.tile([HID, W4], F16, tag="sq", bufs=2,
                                       name=f"hsq_{b}_{g0}")
                        ps = [pp.tile([HID, 2, NT], F32, tag="p",
                                      name=f"hp_{b}_{g0}_{h}") for h in range(2)]
                        for h2 in range(2):
                            c0 = c0q + h2 * W2
                            for k in range(2):
                                cc = c0 + k * NT
                                nc.tensor.matmul(ps[h2][:, k, :], wsq[:, I_QGH, :],
                                                 cur[b][:, cc:cc + NT],
                                                 start=True, stop=True)
                        nc.scalar.activation(c_sb[:, 0:W2], ps[0][:, :, :],
                                             AF.Identity, bias=0.0, scale=1.0)
                        nc.vector.tensor_scalar(
                            out=c_sb[:, W2:W4], in0=ps[1][:, :, :],
                            scalar1=0.0, scalar2=None, op0=ALU.add)
                        nc.vector.tensor_tensor(out=sq[:, :], in0=c_sb[:, :],
                                                in1=c_sb[:, :], op=ALU.mult)
                        v = pv.tile([HID, 4, NT], F32, tag="v",
                                    name=f"hv_{b}_{g0}")
                        for kk in range(4):
                            nc.tensor.matmul(v[:, kk, :], wsq[:, I_R1H, :],
                                             sq[:, kk * NT:(kk + 1) * NT],
                                             start=True, stop=True)
                        nc.scalar.activation(rstd[:, :], v[:, :, :],
                                             AF.Abs_reciprocal_sqrt,
                                             bias=bv(V_EPS), scale=1.0)
                        nc.vector.tensor_tensor(
                            out=zbuf[b][:, c0q:c0q + W4], in0=c_sb[:, :],
                            in1=rstd[:, :], op=ALU.mult)

                # --- head MLP + output (fp16 out) ---
                for b in range(B_LOC):
                    for jq in range(nq):
                        z2s = []
                        for half in range(2):
                            jp = 2 * jq + half
                            c0 = 1 + W2 * jp
                            p = pp.tile([HID, 2, NT], F32, tag="p",
                                        name=f"mp_{b}_{jq}_{half}")
                            for k in range(2):
                                cc = c0 + k * NT
                                nc.tensor.matmul(p[:, k, :], wsq[:, I_HW1, :],
                                                 zbuf[b][:, cc:cc + NT],
                                                 start=True, stop=True)
                            z2 = work.tile([HID, W2], F16, tag=f"z2_{half}",
                                           name=f"z2_{b}_{jq}_{half}")
                            nc.scalar.activation(z2[:, :], p[:, :, :], AF.Gelu,
                                                 bias=bv(V_HB1), scale=1.0)
                            z2s.append(z2)
                        o = pv.tile([HID, NT], F32, tag="v", name=f"o_{b}_{jq}")
                        for k in range(4):
                            zsrc = z2s[k // 2]
                            nc.tensor.matmul(o[32 * k:32 * k + 2, :], w_h2[:, :],
                                             zsrc[:, (k % 2) * NT:(k % 2 + 1) * NT],
                                             start=True, stop=True,
                                             tile_position=(0, 32 * k))
                        osb = work.tile([HID, NT], F16, tag="osb", bufs=2,
                                        name=f"osb_{b}_{jq}")
                        nc.vector.tensor_copy(osb[:, :], o[:, :])
                        og = osb.rearrange("(a b) n -> a b n", b=32)
                        c0g = NT * 4 * jq
                        nc.sync.dma_start(
                            out=out_e[b, :, c0g:c0g + 4 * NT].rearrange(
                                "r (a n) -> a r n", n=NT),
                            in_=og[:, 0:2, :])

    nc.finalize()
    return nc


# ---------------- host pre/post ----------------

def _erf(a):
    try:
        from scipy.special import erf
        return erf(a)
    except Exception:
        import math
        return np.vectorize(math.erf)(a).astype(a.dtype)


def _gelu(a):
    return (0.5 * a * (1.0 + _erf(a / np.sqrt(2.0)))).astype(np.float32)


def host_pre(inputs):
    """Returns (si, x_s, xt): sort perm, sorted x, transposed fp16 x."""
    wl = np.asarray(inputs["wavelengths"])
    si = np.argsort(wl, kind="stable")
    x = np.asarray(inputs["x"])
    x_s = x[:, si, :]                              # [B, C, 8]
    xt = np.ascontiguousarray(x_s.transpose(0, 2, 1)).astype(np.float16)  # [B, 8, C]
    return si, x_s, xt


def host_edge_window(inputs, x_s, b, side, W=32):
    """Exact f32 forward on a window of sorted channels for one batch."""
    deg = np.full(C, 3.0, np.float32)
    deg[0] = deg[-1] = 2.0
    dinv = deg ** -0.5
    idx = np.arange(W) if side == 0 else np.arange(C - W, C)
    xb = x_s[b, idx, :].astype(np.float32)
    dv = dinv[idx]
    A = np.zeros((W, W), np.float32)
    for j in range(W):
        A[j, j] = dv[j] * dv[j]
        if j > 0:
            A[j, j - 1] = dv[j] * dv[j - 1]
        if j < W - 1:
            A[j, j + 1] = dv[j] * dv[j + 1]
    h = _gelu(xb @ inputs["in_w1"] + inputs["in_b1"]) @ inputs["in_w2"] + inputs["in_b2"]
    h = h.astype(np.float32)
    for l in range(L):
        hl = h @ inputs["gcn_w"][l] + inputs["gcn_b"][l]
        s = A @ hl + h
        mu_ = s.mean(-1, keepdims=True)
        var_ = ((s - mu_) ** 2).mean(-1, keepdims=True)
        ln = (s - mu_) / np.sqrt(var_ + EPS) * inputs["gcn_g"][l] + inputs["gcn_beta"][l]
        h = np.maximum(ln, 0.0)
    mu_ = h.mean(-1, keepdims=True)
    var_ = ((h - mu_) ** 2).mean(-1, keepdims=True)
    z = (h - mu_) / np.sqrt(var_ + EPS) * inputs["head_ln_g"] + inputs["head_ln_b"]
    z = _gelu(z @ inputs["head_w1"] + inputs["head_b1"])
    o = z @ inputs["head_w2"] + inputs["head_b2"]
    mu = o[:, 0]
    ls = np.clip(o[:, 1], MIN_LS, MAX_LS)
    return np.stack([mu, ls], -1), idx


def host_post(inputs, si, x_s, dev_outs, FIX=16, W=32):
    """dev_outs: list of per-core {'out': [B_LOC, 2, C]} → full [B, C, 2] f32."""
    b2 = np.asarray(inputs["head_b2"]).astype(np.float32)
    out_s = np.zeros((B, C, 2), np.float32)
    for core in range(N_CORES):
        for bl in range(B_LOC):
            b = core * B_LOC + bl
            o = dev_outs[core]["out"][bl].astype(np.float32)   # [2, C]
            mu = o[0] + b2[0]
            ls = np.clip(o[1] + b2[1], MIN_LS, MAX_LS)
            out_s[b] = np.stack([mu, ls], -1)
    inp_np = {k: np.asarray(v) for k, v in inputs.items()}
    for b in range(B):
        for side in (0, 1):
            ow, idx = host_edge_window(inp_np, x_s, b, side, W)
            if side == 0:
                out_s[b, idx[:FIX]] = ow[:FIX]
            else:
                out_s[b, idx[-FIX:]] = ow[-FIX:]
    out = np.zeros_like(out_s)
    out[:, si, :] = out_s
    return out


# ---------------- kernel entry point ----------------

_CACHE = {}
# build_nc (the original engine-balanced build) wins in the timeline simulator
# over every build_nc2 variant tried (ACT-rebalanced, S-merged stencil, fused
# relu*rstd, quad-wide rsqrt): 392 us vs 494-518 us — its schedule overlaps
# engines at ~91% of the max-engine busy time, which the variants all break.
BUILD = build_nc


def _get_nc(inputs):
    fuse = (not np.any(np.asarray(inputs["gcn_beta"]))) if BUILD is build_nc2 else None
    key = ("nc", fuse)
    if key not in _CACHE:
        _CACHE[key] = build_nc2(fuse_relu=fuse) if BUILD is build_nc2 else build_nc()
    return _CACHE[key]


def kernel(**inputs) -> np.ndarray:
    from concourse.bass_utils import run_bass_kernel_spmd

    inputs = {k: np.asarray(v) for k, v in inputs.items()}
    nc = _get_nc(inputs)
    si, x_s, xt = host_pre(inputs)
    wsq, biasv, w_in1, w_h2 = pack_host(inputs)
    in_maps = []
    for core in range(N_CORES):
        in_maps.append({
            "xt": np.ascontiguousarray(xt[core * B_LOC:(core + 1) * B_LOC]),
            "wsq": wsq, "biasv": biasv, "w_in1": w_in1, "w_h2": w_h2,
        })
    res = run_bass_kernel_spmd(nc, in_maps, core_ids=list(range(N_CORES)))
    return host_post(inputs, si, x_s, res.results)

